# revision 12
# baseline (speedup 1.0000x reference)
"""MoE ExpertPool kernel for 8 Trainium2 NeuronCores (expert-parallel).

Host side: one expert per core.  Tokens routed to expert e (via either
top-k slot) are gathered and padded to a common capacity C (multiple of
64).  All device tensors are pre-arranged on the host so every DMA is
contiguous per partition and every matmul uses natural [K, M] layouts:

  device (per core):  H = silu(Wg^T @ xT) * (Wu^T @ xT)      [d_expert, C]
                      yT = Wd^T @ H                          [d_model, C]

Activations stay transposed ([feature, token]) the whole way, so the
tokens live on the matmul free dim and weights are the stationary lhsT.
The per-token routing weight and the scatter-add back to (B,S,D) happen
on the host (they are linear post-ops of yT).

All matmul operands are bf16 (fp32 PSUM accumulate): full PE rate, half
the HBM traffic / SBUF footprint of fp32r, and lower PE power (less
DVFS throttling).  End-to-end absmax-relative error ~4e-3.

Token chunks are C/2 wide (<= 512), so there is no ragged tail: the
gate/up pipeline keeps 8 PSUM banks (4 gate + 4 up) open and every
matmul runs at the full 1 col/cycle bf16 rate.
"""

import numpy as np

D_MODEL = 768
D_EXPERT = 3072
N_EXPERTS = 8
TOP_K = 2
P = 128
KD = D_MODEL // P      # 6   d_model chunks of 128
MD = D_EXPERT // P     # 24  d_expert chunks of 128
WG_W = 256             # gate/up stationary-weight tile width
N_WG = D_EXPERT // WG_W

_CACHE = {}
LAST_RESULTS = None


def _ensure_axon_hooks():
    """Provide antenv.axon_hooks if the image lacks it, so the trace=True
    path of run_bass_kernel_spmd works (and BASS_TRACE=1 can't crash us)."""
    import sys
    import types

    try:
        import antenv.axon_hooks  # noqa: F401

        return
    except ImportError:
        pass
    try:
        import antenv
    except ImportError:
        return
    mod = types.ModuleType("antenv.axon_hooks")
    mod._hook = None
    mod.set_axon_ntff_profile_hook = lambda h: setattr(mod, "_hook", h)
    mod.get_axon_ntff_profile_hook = lambda: mod._hook
    sys.modules["antenv.axon_hooks"] = mod
    antenv.axon_hooks = mod
    try:
        from trn_agent_boot.trn_boot import _ntff_profile_via_ctypes

        hook = _ntff_profile_via_ctypes("/opt/axon/libaxon_pjrt.so")
        if hook is not None:
            mod._hook = hook
    except Exception:
        pass


def _build(C):
    import concourse.mybir as mybir
    import concourse.tile as tile
    from concourse import bacc

    f32 = mybir.dt.float32
    bf16 = mybir.dt.bfloat16
    Act = mybir.ActivationFunctionType

    TCH = C // 2           # token chunk; C in [512,1024] -> TCH in [256,512]
    NB = 2
    assert TCH * NB == C and 256 <= TCH <= 512

    nc = bacc.Bacc("TRN2", dynamic_dma_scratch_size=512, num_swdge_queues=1)
    xt = nc.dram_tensor("xt", [P, KD, C], bf16, kind="ExternalInput")
    wg = nc.dram_tensor("wg", [P, N_WG, KD, WG_W], bf16, kind="ExternalInput")
    wu = nc.dram_tensor("wu", [P, N_WG, KD, WG_W], bf16, kind="ExternalInput")
    wd = nc.dram_tensor("wd", [P, KD, MD, P], bf16, kind="ExternalInput")
    yt = nc.dram_tensor("yt", [P, KD, C], f32, kind="ExternalOutput")

    with tile.TileContext(nc) as tc:
        with (
            tc.tile_pool(name="singles", bufs=1) as singles,
            tc.tile_pool(name="wpool", bufs=2) as wpool,
            tc.tile_pool(name="tmp", bufs=3) as tmp,
            tc.tile_pool(name="psum", bufs=2, space="PSUM") as psum,
        ):
            xt_sb = singles.tile([P, KD, C], bf16)
            H_sb = singles.tile([P, MD, C], bf16)

            # PE pre-warm: dummy matmuls on a zeroed tile while the first
            # DMAs are in flight, so the PE p-state ramp runs during the DMA
            # window and the PE is near full clock when real matmuls start.
            warm_sb = singles.tile([P, 512], bf16, name="warm_sb")
            nc.vector.memset(warm_sb[:], 0.0)
            warm_ps = psum.tile([P, 512], f32, tag="ups", bufs=4,
                                name="warm_ps")
            # Small matmuls pace the warmup finely: if the first operands
            # land mid-warmup only ~0.1us is wasted, and the PE busy-ramp
            # timer keeps running right up to the first real matmul.
            for _ in range(34):
                nc.tensor.matmul(
                    warm_ps[:, :64], warm_sb[:, :P], warm_sb[:, :64],
                    start=True, stop=True
                )

            # Prologue: xt k-chunks alternate between the two HWDGE queues
            # (SP + ACT), interleaved with the mo=0 weight chunks, so the
            # first matmul starts right after the framework preamble and the
            # m=0 k-loop is fed at DMA pace without enqueue-rate overhead.
            wg_t0 = wpool.tile([P, KD, WG_W], bf16, tag="wg", bufs=3, name="wg_t0")
            wu_t0 = wpool.tile([P, KD, WG_W], bf16, tag="wu", bufs=3, name="wu_t0")
            # The k=0 xt chunk is split in half across both queues so the
            # very first gate matmul (m=0, b=0: needs xt[:,0,:TCH] + wg0[k=0]
            # only) has its operands ~1.4us earlier than a whole-chunk load.
            sync_q = [(xt_sb[:, 0, :TCH], xt[:, 0, :TCH]),
                      (wu_t0[:, 0], wu[:, 0, 0])]
            scal_q = [(wg_t0[:, 0], wg[:, 0, 0]),
                      (xt_sb[:, 0, TCH:], xt[:, 0, TCH:])]
            for k in range(1, KD):
                (sync_q if k % 2 == 0 else scal_q).append(
                    (xt_sb[:, k], xt[:, k]))
                (scal_q if k % 2 == 0 else sync_q).append(
                    (wg_t0[:, k], wg[:, 0, k]))
                (sync_q if k % 2 == 0 else scal_q).append(
                    (wu_t0[:, k], wu[:, 0, k]))
            for eng, q in ((nc.sync, sync_q), (nc.scalar, scal_q)):
                for dst, srcap in q:
                    eng.dma_start(out=dst, in_=srcap)

            # First two m-groups interleave gate and up per k, paced by the
            # chunk arrivals above; their 8 PSUM groups stay open through the
            # whole xt load so the PE does real work during the DMA window.
            part = []
            for mj in range(2):
                m = mj
                ms = slice(mj * P, (mj + 1) * P)
                g_ps = [
                    psum.tile([P, TCH], f32, tag="gps", bufs=4,
                              name=f"g_{m}_{b}")
                    for b in range(NB)
                ]
                u_ps = [
                    psum.tile([P, TCH], f32, tag="ups", bufs=4,
                              name=f"u_{m}_{b}")
                    for b in range(NB)
                ]
                part.append((m, ms, g_ps, u_ps))
            for k in range(KD):
                st, sp = k == 0, k == KD - 1
                for b in range(NB):
                    for m, ms, g_ps, u_ps in part:
                        nc.tensor.matmul(
                            g_ps[b], wg_t0[:, k, ms],
                            xt_sb[:, k, b * TCH : (b + 1) * TCH],
                            start=st, stop=sp,
                        )
                for b in range(NB):
                    for m, ms, g_ps, u_ps in part:
                        nc.tensor.matmul(
                            u_ps[b], wu_t0[:, k, ms],
                            xt_sb[:, k, b * TCH : (b + 1) * TCH],
                            start=st, stop=sp,
                        )
            for m, ms, g_ps, u_ps in part:
                sils = []
                for b in range(NB):
                    sil = tmp.tile([P, TCH], f32, tag="sil", bufs=2,
                                   name=f"sil_p{m}_{b}")
                    nc.scalar.activation(out=sil[:], in_=g_ps[b], func=Act.Silu)
                    sils.append(sil)
                for b in range(NB):
                    nc.vector.tensor_mul(
                        H_sb[:, m, b * TCH : (b + 1) * TCH], sils[b], u_ps[b]
                    )

            # gate/up projections + silu*mul -> H   (d_expert = m*128 + p).
            for mo in range(N_WG):
                if mo == 0:
                    wg_t, wu_t = wg_t0, wu_t0
                else:
                    wg_t = wpool.tile([P, KD, WG_W], bf16, tag="wg", bufs=3)
                    nc.sync.dma_start(out=wg_t[:], in_=wg[:, mo])
                    wu_t = wpool.tile([P, KD, WG_W], bf16, tag="wu", bufs=3)
                    nc.scalar.dma_start(out=wu_t[:], in_=wu[:, mo])
                for mj in range(WG_W // P):
                    m = mo * (WG_W // P) + mj
                    if m < 2:
                        continue
                    ms = slice(mj * P, (mj + 1) * P)
                    g_ps = [
                        psum.tile([P, TCH], f32, tag="gps", bufs=4,
                                  name=f"g_{m}_{b}")
                        for b in range(NB)
                    ]
                    u_ps = [
                        psum.tile([P, TCH], f32, tag="ups", bufs=4,
                                  name=f"u_{m}_{b}")
                        for b in range(NB)
                    ]
                    for k in range(KD):
                        st, sp = k == 0, k == KD - 1
                        for b in range(NB):
                            nc.tensor.matmul(
                                g_ps[b],
                                wg_t[:, k, ms],
                                xt_sb[:, k, b * TCH : (b + 1) * TCH],
                                start=st, stop=sp,
                            )
                    sils = []
                    for b in range(NB):
                        sil = tmp.tile([P, TCH], f32, tag="sil", bufs=2,
                                       name=f"sil_{m}_{b}")
                        nc.scalar.activation(out=sil[:], in_=g_ps[b], func=Act.Silu)
                        sils.append(sil)
                    for k in range(KD):
                        st, sp = k == 0, k == KD - 1
                        for b in range(NB):
                            nc.tensor.matmul(
                                u_ps[b],
                                wu_t[:, k, ms],
                                xt_sb[:, k, b * TCH : (b + 1) * TCH],
                                start=st, stop=sp,
                            )
                    for b in range(NB):
                        nc.vector.tensor_mul(
                            H_sb[:, m, b * TCH : (b + 1) * TCH], sils[b], u_ps[b]
                        )

            # down projection   (d_model = n*128 + p); reuses the gps PSUM
            # tag so the kernel stays within 8 banks.
            MDH = MD // 2
            QC = C // 4
            for n in range(KD):
                wd_h = []
                for h in range(2):
                    wd_t = wpool.tile(
                        [P, MDH, P], bf16, tag="wd", bufs=3, name=f"wd_{n}_{h}"
                    )
                    eng = nc.sync if h == 0 else nc.scalar
                    eng.dma_start(
                        out=wd_t[:], in_=wd[:, n, h * MDH : (h + 1) * MDH]
                    )
                    wd_h.append(wd_t)
                last = n == KD - 1
                # Last n-group is the kernel's critical tail: use 4 quarter
                # groups so the final copies+DMAs are small and fan out over
                # four engines/queues in parallel.
                nq = 4 if last else NB
                qw = QC if last else TCH
                y_ps = [
                    psum.tile([P, qw], f32, tag="gps", bufs=4,
                              name=f"y_{n}_{b}")
                    for b in range(nq)
                ]
                for k in range(MD):
                    st, sp = k == 0, k == MD - 1
                    lhs = wd_h[k // MDH][:, k % MDH, :]
                    for b in range(nq):
                        nc.tensor.matmul(
                            y_ps[b],
                            lhs,
                            H_sb[:, k, b * qw : (b + 1) * qw],
                            start=st, stop=sp,
                        )
                if last:
                    # Quarter groups drain into one SBUF tile via both copy
                    # engines; each HWDGE queue then ships one half in a
                    # single enqueue, so the post-matmul chain is
                    # copy(0.4us) -> enqueue(0.6) -> transfer+sem.
                    y_sb = tmp.tile([P, C], f32, tag="ysbl", bufs=1,
                                    name="ysbl")
                    cps = [nc.vector, nc.scalar, nc.vector, nc.scalar]
                    for q in range(4):
                        qs = slice(q * QC, (q + 1) * QC)
                        if cps[q] is nc.scalar:
                            cps[q].copy(out=y_sb[:, qs], in_=y_ps[q])
                        else:
                            cps[q].tensor_copy(out=y_sb[:, qs], in_=y_ps[q])
                    for h, eng in ((0, nc.sync), (1, nc.scalar)):
                        hs = slice(h * TCH, (h + 1) * TCH)
                        eng.dma_start(out=yt[:, n, hs], in_=y_sb[:, hs])
                else:
                    for b in range(NB):
                        y_sb = tmp.tile([P, TCH], f32, tag="ysb", bufs=2,
                                        name=f"ysb_{n}_{b}")
                        nc.any.tensor_copy(out=y_sb[:], in_=y_ps[b])
                        (nc.sync if b % 2 == 0 else nc.scalar).dma_start(
                            out=yt[:, n, b * TCH : (b + 1) * TCH], in_=y_sb[:]
                        )
    nc.finalize()
    return nc


def kernel(**inputs):
    global LAST_RESULTS
    import ml_dtypes

    bf16 = ml_dtypes.bfloat16
    x = np.ascontiguousarray(np.asarray(inputs["x"], dtype=np.float32))
    rw = np.asarray(inputs["routing_weights"], dtype=np.float32)
    ei = np.asarray(inputs["expert_indices"])
    wg = np.asarray(inputs["w_gate"], dtype=np.float32)
    wu = np.asarray(inputs["w_up"], dtype=np.float32)
    wd = np.asarray(inputs["w_down"], dtype=np.float32)

    B, S, D = x.shape
    T = B * S
    xf = x.reshape(T, D)
    eif = ei.reshape(T, TOP_K).astype(np.int64)
    rwf = rw.reshape(T, TOP_K)

    # per-token weight for each expert (sum over top-k slots assigned to e)
    tokw = np.zeros((T, N_EXPERTS), np.float32)
    np.add.at(tokw, (np.arange(T)[:, None], eif), rwf)

    idxs = [np.nonzero((eif == e).any(axis=1))[0] for e in range(N_EXPERTS)]
    # Capacity: smallest multiple of 64 in [512, 1024] that spills at most
    # ~2% of routed tokens to the (exact) host path — streamed columns are
    # the dominant device cost, so C directly scales kernel time.  Capped at
    # 1024 so xt+H stay within SBUF.
    routed = sum(len(i) for i in idxs)
    budget = max(P, routed * 2 // 100)
    C = 1024
    for cand in range(512, 1025, 64):
        if sum(max(0, len(i) - cand) for i in idxs) <= budget:
            C = cand
            break

    _ensure_axon_hooks()
    from concourse.bass_utils import run_bass_kernel_spmd

    nc = _CACHE.get(C)
    if nc is None:
        nc = _CACHE[C] = _build(C)

    wg_b = wg.astype(bf16)
    wu_b = wu.astype(bf16)
    wd_b = wd.astype(bf16)
    in_maps = []
    for e in range(N_EXPERTS):
        idx = idxs[e][:C]
        xe = np.zeros((C, D), np.float32)
        xe[: len(idx)] = xf[idx]
        in_maps.append(
            {
                "xt": np.ascontiguousarray(
                    xe.T.reshape(KD, P, C).transpose(1, 0, 2).astype(bf16)
                ),
                "wg": np.ascontiguousarray(
                    wg_b[e].reshape(KD, P, N_WG, WG_W).transpose(1, 2, 0, 3)
                ),
                "wu": np.ascontiguousarray(
                    wu_b[e].reshape(KD, P, N_WG, WG_W).transpose(1, 2, 0, 3)
                ),
                "wd": np.ascontiguousarray(
                    wd_b[e].reshape(MD, P, KD, P).transpose(1, 2, 0, 3)
                ),
            }
        )

    try:
        res = run_bass_kernel_spmd(nc, in_maps, core_ids=list(range(N_EXPERTS)))
    except Exception:
        # transient NRT/device hiccups (e.g. NRT_EXEC_UNIT_UNRECOVERABLE)
        # usually clear on a retry
        res = run_bass_kernel_spmd(nc, in_maps, core_ids=list(range(N_EXPERTS)))
    LAST_RESULTS = res

    out = np.zeros((T, D), np.float32)
    for e in range(N_EXPERTS):
        idx = idxs[e][:C]
        ye = res.results[e]["yt"].transpose(1, 0, 2).reshape(D, C).T
        out[idx] += ye[: len(idx)] * tokw[idx, e][:, None]
        spill = idxs[e][C:]
        if len(spill):
            xs = xf[spill]
            h = xs @ wg[e]
            h = (h / (1.0 + np.exp(-h))) * (xs @ wu[e])
            out[spill] += (h @ wd[e]) * tokw[spill, e][:, None]
    return out.reshape(B, S, D)


# revision 13
# speedup vs baseline: 1.0131x; 1.0131x over previous
"""MoE ExpertPool kernel for 8 Trainium2 NeuronCores (expert-parallel).

Host side: one expert per core.  Tokens routed to expert e (via either
top-k slot) are gathered and padded to a common capacity C (multiple of
64).  All device tensors are pre-arranged on the host so every DMA is
contiguous per partition and every matmul uses natural [K, M] layouts:

  device (per core):  H = silu(Wg^T @ xT) * (Wu^T @ xT)      [d_expert, C]
                      yT = Wd^T @ H                          [d_model, C]

Activations stay transposed ([feature, token]) the whole way, so the
tokens live on the matmul free dim and weights are the stationary lhsT.
The per-token routing weight and the scatter-add back to (B,S,D) happen
on the host (they are linear post-ops of yT).

All matmul operands are bf16 (fp32 PSUM accumulate): full PE rate, half
the HBM traffic / SBUF footprint of fp32r, and lower PE power (less
DVFS throttling).  End-to-end absmax-relative error ~4e-3.

Token chunks are C/2 wide (<= 512), so there is no ragged tail: the
gate/up pipeline keeps 8 PSUM banks (4 gate + 4 up) open and every
matmul runs at the full 1 col/cycle bf16 rate.
"""

import numpy as np

D_MODEL = 768
D_EXPERT = 3072
N_EXPERTS = 8
TOP_K = 2
P = 128
KD = D_MODEL // P      # 6   d_model chunks of 128
MD = D_EXPERT // P     # 24  d_expert chunks of 128
WG_W = 256             # gate/up stationary-weight tile width
N_WG = D_EXPERT // WG_W

_CACHE = {}
LAST_RESULTS = None


def _ensure_axon_hooks():
    """Provide antenv.axon_hooks if the image lacks it, so the trace=True
    path of run_bass_kernel_spmd works (and BASS_TRACE=1 can't crash us)."""
    import sys
    import types

    try:
        import antenv.axon_hooks  # noqa: F401

        return
    except ImportError:
        pass
    try:
        import antenv
    except ImportError:
        return
    mod = types.ModuleType("antenv.axon_hooks")
    mod._hook = None
    mod.set_axon_ntff_profile_hook = lambda h: setattr(mod, "_hook", h)
    mod.get_axon_ntff_profile_hook = lambda: mod._hook
    sys.modules["antenv.axon_hooks"] = mod
    antenv.axon_hooks = mod
    try:
        from trn_agent_boot.trn_boot import _ntff_profile_via_ctypes

        hook = _ntff_profile_via_ctypes("/opt/axon/libaxon_pjrt.so")
        if hook is not None:
            mod._hook = hook
    except Exception:
        pass


def _build(C):
    import concourse.mybir as mybir
    import concourse.tile as tile
    from concourse import bacc

    f32 = mybir.dt.float32
    bf16 = mybir.dt.bfloat16
    Act = mybir.ActivationFunctionType

    TCH = C // 2           # token chunk; C in [512,1024] -> TCH in [256,512]
    NB = 2
    assert TCH * NB == C and 256 <= TCH <= 512

    nc = bacc.Bacc("TRN2", dynamic_dma_scratch_size=512, num_swdge_queues=1)
    xt = nc.dram_tensor("xt", [P, KD, C], bf16, kind="ExternalInput")
    wg = nc.dram_tensor("wg", [P, N_WG, KD, WG_W], bf16, kind="ExternalInput")
    wu = nc.dram_tensor("wu", [P, N_WG, KD, WG_W], bf16, kind="ExternalInput")
    wd = nc.dram_tensor("wd", [P, KD, MD, P], bf16, kind="ExternalInput")
    yt = nc.dram_tensor("yt", [P, KD, C], f32, kind="ExternalOutput")

    with tile.TileContext(nc) as tc:
        with (
            tc.tile_pool(name="singles", bufs=1) as singles,
            tc.tile_pool(name="wpool", bufs=2) as wpool,
            tc.tile_pool(name="tmp", bufs=3) as tmp,
            tc.tile_pool(name="psum", bufs=2, space="PSUM") as psum,
        ):
            xt_sb = singles.tile([P, KD, C], bf16)
            H_sb = singles.tile([P, MD, C], bf16)

            # PE pre-warm: dummy matmuls on a zeroed tile while the first
            # DMAs are in flight, so the PE p-state ramp runs during the DMA
            # window and the PE is near full clock when real matmuls start.
            warm_sb = singles.tile([P, 512], bf16, name="warm_sb")
            nc.vector.memset(warm_sb[:], 0.0)
            warm_ps = psum.tile([P, 512], f32, tag="ups", bufs=4,
                                name="warm_ps")
            # Small matmuls pace the warmup finely: if the first operands
            # land mid-warmup only ~0.1us is wasted, and the PE busy-ramp
            # timer keeps running right up to the first real matmul.
            for _ in range(48):
                nc.tensor.matmul(
                    warm_ps[:, :64], warm_sb[:, :P], warm_sb[:, :64],
                    start=True, stop=True
                )

            # Prologue: xt k-chunks alternate between the two HWDGE queues
            # (SP + ACT), interleaved with the mo=0 weight chunks, so the
            # first matmul starts right after the framework preamble and the
            # m=0 k-loop is fed at DMA pace without enqueue-rate overhead.
            wg_t0 = wpool.tile([P, KD, WG_W], bf16, tag="wg", bufs=3, name="wg_t0")
            wu_t0 = wpool.tile([P, KD, WG_W], bf16, tag="wu", bufs=3, name="wu_t0")
            qrr = [nc.sync, nc.scalar]
            qi = 0
            for k in range(KD):
                for dst, srcap in (
                    (xt_sb[:, k], xt[:, k]),
                    (wg_t0[:, k], wg[:, 0, k]),
                    (wu_t0[:, k], wu[:, 0, k]),
                ):
                    qrr[qi % 2].dma_start(out=dst, in_=srcap)
                    qi += 1

            # First two m-groups interleave gate and up per k, paced by the
            # chunk arrivals above; their 8 PSUM groups stay open through the
            # whole xt load so the PE does real work during the DMA window.
            part = []
            for mj in range(2):
                m = mj
                ms = slice(mj * P, (mj + 1) * P)
                g_ps = [
                    psum.tile([P, TCH], f32, tag="gps", bufs=4,
                              name=f"g_{m}_{b}")
                    for b in range(NB)
                ]
                u_ps = [
                    psum.tile([P, TCH], f32, tag="ups", bufs=4,
                              name=f"u_{m}_{b}")
                    for b in range(NB)
                ]
                part.append((m, ms, g_ps, u_ps))
            for k in range(KD):
                st, sp = k == 0, k == KD - 1
                for m, ms, g_ps, u_ps in part:
                    for b in range(NB):
                        nc.tensor.matmul(
                            g_ps[b], wg_t0[:, k, ms],
                            xt_sb[:, k, b * TCH : (b + 1) * TCH],
                            start=st, stop=sp,
                        )
                for m, ms, g_ps, u_ps in part:
                    for b in range(NB):
                        nc.tensor.matmul(
                            u_ps[b], wu_t0[:, k, ms],
                            xt_sb[:, k, b * TCH : (b + 1) * TCH],
                            start=st, stop=sp,
                        )
            for m, ms, g_ps, u_ps in part:
                sils = []
                for b in range(NB):
                    sil = tmp.tile([P, TCH], f32, tag="sil", bufs=2,
                                   name=f"sil_p{m}_{b}")
                    nc.scalar.activation(out=sil[:], in_=g_ps[b], func=Act.Silu)
                    sils.append(sil)
                for b in range(NB):
                    nc.vector.tensor_mul(
                        H_sb[:, m, b * TCH : (b + 1) * TCH], sils[b], u_ps[b]
                    )

            # gate/up projections + silu*mul -> H   (d_expert = m*128 + p).
            for mo in range(N_WG):
                if mo == 0:
                    wg_t, wu_t = wg_t0, wu_t0
                else:
                    wg_t = wpool.tile([P, KD, WG_W], bf16, tag="wg", bufs=3)
                    nc.sync.dma_start(out=wg_t[:], in_=wg[:, mo])
                    wu_t = wpool.tile([P, KD, WG_W], bf16, tag="wu", bufs=3)
                    nc.scalar.dma_start(out=wu_t[:], in_=wu[:, mo])
                for mj in range(WG_W // P):
                    m = mo * (WG_W // P) + mj
                    if m < 2:
                        continue
                    ms = slice(mj * P, (mj + 1) * P)
                    g_ps = [
                        psum.tile([P, TCH], f32, tag="gps", bufs=4,
                                  name=f"g_{m}_{b}")
                        for b in range(NB)
                    ]
                    u_ps = [
                        psum.tile([P, TCH], f32, tag="ups", bufs=4,
                                  name=f"u_{m}_{b}")
                        for b in range(NB)
                    ]
                    for k in range(KD):
                        st, sp = k == 0, k == KD - 1
                        for b in range(NB):
                            nc.tensor.matmul(
                                g_ps[b],
                                wg_t[:, k, ms],
                                xt_sb[:, k, b * TCH : (b + 1) * TCH],
                                start=st, stop=sp,
                            )
                    sils = []
                    for b in range(NB):
                        sil = tmp.tile([P, TCH], f32, tag="sil", bufs=2,
                                       name=f"sil_{m}_{b}")
                        nc.scalar.activation(out=sil[:], in_=g_ps[b], func=Act.Silu)
                        sils.append(sil)
                    for k in range(KD):
                        st, sp = k == 0, k == KD - 1
                        for b in range(NB):
                            nc.tensor.matmul(
                                u_ps[b],
                                wu_t[:, k, ms],
                                xt_sb[:, k, b * TCH : (b + 1) * TCH],
                                start=st, stop=sp,
                            )
                    for b in range(NB):
                        nc.vector.tensor_mul(
                            H_sb[:, m, b * TCH : (b + 1) * TCH], sils[b], u_ps[b]
                        )

            # down projection   (d_model = n*128 + p); reuses the gps PSUM
            # tag so the kernel stays within 8 banks.
            MDH = MD // 2
            QC = C // 4
            for n in range(KD):
                wd_h = []
                for h in range(2):
                    wd_t = wpool.tile(
                        [P, MDH, P], bf16, tag="wd", bufs=3, name=f"wd_{n}_{h}"
                    )
                    eng = nc.sync if h == 0 else nc.scalar
                    eng.dma_start(
                        out=wd_t[:], in_=wd[:, n, h * MDH : (h + 1) * MDH]
                    )
                    wd_h.append(wd_t)
                last = n == KD - 1
                # Last n-group is the kernel's critical tail: use 4 quarter
                # groups so the final copies+DMAs are small and fan out over
                # four engines/queues in parallel.
                nq = 4 if last else NB
                qw = QC if last else TCH
                y_ps = [
                    psum.tile([P, qw], f32, tag="gps", bufs=4,
                              name=f"y_{n}_{b}")
                    for b in range(nq)
                ]
                for k in range(MD):
                    st, sp = k == 0, k == MD - 1
                    lhs = wd_h[k // MDH][:, k % MDH, :]
                    for b in range(nq):
                        nc.tensor.matmul(
                            y_ps[b],
                            lhs,
                            H_sb[:, k, b * qw : (b + 1) * qw],
                            start=st, stop=sp,
                        )
                if last:
                    # Quarter groups drain into one SBUF tile via both copy
                    # engines; each HWDGE queue then ships one half in a
                    # single enqueue, so the post-matmul chain is
                    # copy(0.4us) -> enqueue(0.6) -> transfer+sem.
                    y_sb = tmp.tile([P, C], f32, tag="ysbl", bufs=1,
                                    name="ysbl")
                    cps = [nc.vector, nc.scalar, nc.vector, nc.scalar]
                    for q in range(4):
                        qs = slice(q * QC, (q + 1) * QC)
                        if cps[q] is nc.scalar:
                            cps[q].copy(out=y_sb[:, qs], in_=y_ps[q])
                        else:
                            cps[q].tensor_copy(out=y_sb[:, qs], in_=y_ps[q])
                    for h, eng in ((0, nc.sync), (1, nc.scalar)):
                        hs = slice(h * TCH, (h + 1) * TCH)
                        eng.dma_start(out=yt[:, n, hs], in_=y_sb[:, hs])
                else:
                    for b in range(NB):
                        y_sb = tmp.tile([P, TCH], f32, tag="ysb", bufs=2,
                                        name=f"ysb_{n}_{b}")
                        nc.any.tensor_copy(out=y_sb[:], in_=y_ps[b])
                        (nc.sync if b % 2 == 0 else nc.scalar).dma_start(
                            out=yt[:, n, b * TCH : (b + 1) * TCH], in_=y_sb[:]
                        )
    nc.finalize()
    return nc


def kernel(**inputs):
    global LAST_RESULTS
    import ml_dtypes

    bf16 = ml_dtypes.bfloat16
    x = np.ascontiguousarray(np.asarray(inputs["x"], dtype=np.float32))
    rw = np.asarray(inputs["routing_weights"], dtype=np.float32)
    ei = np.asarray(inputs["expert_indices"])
    wg = np.asarray(inputs["w_gate"], dtype=np.float32)
    wu = np.asarray(inputs["w_up"], dtype=np.float32)
    wd = np.asarray(inputs["w_down"], dtype=np.float32)

    B, S, D = x.shape
    T = B * S
    xf = x.reshape(T, D)
    eif = ei.reshape(T, TOP_K).astype(np.int64)
    rwf = rw.reshape(T, TOP_K)

    # per-token weight for each expert (sum over top-k slots assigned to e)
    tokw = np.zeros((T, N_EXPERTS), np.float32)
    np.add.at(tokw, (np.arange(T)[:, None], eif), rwf)

    idxs = [np.nonzero((eif == e).any(axis=1))[0] for e in range(N_EXPERTS)]
    # Capacity: smallest multiple of 64 in [512, 1024] that spills at most
    # ~2% of routed tokens to the (exact) host path — streamed columns are
    # the dominant device cost, so C directly scales kernel time.  Capped at
    # 1024 so xt+H stay within SBUF.
    routed = sum(len(i) for i in idxs)
    budget = max(P, routed * 2 // 100)
    C = 1024
    for cand in range(512, 1025, 64):
        if sum(max(0, len(i) - cand) for i in idxs) <= budget:
            C = cand
            break

    _ensure_axon_hooks()
    from concourse.bass_utils import run_bass_kernel_spmd

    nc = _CACHE.get(C)
    if nc is None:
        nc = _CACHE[C] = _build(C)

    wg_b = wg.astype(bf16)
    wu_b = wu.astype(bf16)
    wd_b = wd.astype(bf16)
    in_maps = []
    for e in range(N_EXPERTS):
        idx = idxs[e][:C]
        xe = np.zeros((C, D), np.float32)
        xe[: len(idx)] = xf[idx]
        in_maps.append(
            {
                "xt": np.ascontiguousarray(
                    xe.T.reshape(KD, P, C).transpose(1, 0, 2).astype(bf16)
                ),
                "wg": np.ascontiguousarray(
                    wg_b[e].reshape(KD, P, N_WG, WG_W).transpose(1, 2, 0, 3)
                ),
                "wu": np.ascontiguousarray(
                    wu_b[e].reshape(KD, P, N_WG, WG_W).transpose(1, 2, 0, 3)
                ),
                "wd": np.ascontiguousarray(
                    wd_b[e].reshape(MD, P, KD, P).transpose(1, 2, 0, 3)
                ),
            }
        )

    try:
        res = run_bass_kernel_spmd(nc, in_maps, core_ids=list(range(N_EXPERTS)))
    except Exception:
        # transient NRT/device hiccups (e.g. NRT_EXEC_UNIT_UNRECOVERABLE)
        # usually clear on a retry
        res = run_bass_kernel_spmd(nc, in_maps, core_ids=list(range(N_EXPERTS)))
    LAST_RESULTS = res

    out = np.zeros((T, D), np.float32)
    for e in range(N_EXPERTS):
        idx = idxs[e][:C]
        ye = res.results[e]["yt"].transpose(1, 0, 2).reshape(D, C).T
        out[idx] += ye[: len(idx)] * tokw[idx, e][:, None]
        spill = idxs[e][C:]
        if len(spill):
            xs = xf[spill]
            h = xs @ wg[e]
            h = (h / (1.0 + np.exp(-h))) * (xs @ wu[e])
            out[spill] += (h @ wd[e]) * tokw[spill, e][:, None]
    return out.reshape(B, S, D)
